# revision 27
# baseline (speedup 1.0000x reference)
"""Trainium2 Bass kernel for nn_Attention_38130719654026 (sparse_attention).

The reference collapses exactly (verified to 9e-8 rel err): the top-k gather
broadcasts kv over the topk axis, so attention logits are constant along it,
softmax is uniform, and attn @ v_sel returns v unchanged.  grad/q/k are dead.
What remains is:

    g   = gelu(x @ W_v)              with W_v = W_qkv[:, 1024:1536]
    y[b, P, n, 64H:64H+64] = g[b, 2H + P//8, n, 64*(P%8) : 64*(P%8)+64]
    out = y @ W_out + b_out

Sharding (8 cores, no collectives): core c -> (batch b = c//4,
window parity q = (c//2)%2, token half t = c%2).

Measured HW model (perfetto traces across 4 kernel revisions):
  - each HWDGE queue sustains ~160 GB/s for >=2KB lines; sub-2KB lines are
    descriptor-bound (~12.6 ns/desc).  Every full-partition piece costs
    >=128 descs.  The 16 SDMA engines round-robin between ACTIVE queues at
    packet granularity, so a third queue (gpsimd SWDGE ~46 GB/s) or even a
    1-desc warmup DMA slows the critical stream -- both tested and reverted.
  - per-piece completion semaphores straggle 0.4-1.6 us behind the bulk of
    the data (16 per-engine increments, last ones delayed by engine
    round-robin).
  - the PE re-enters clock gating after ~1 us idle, then runs ~6 matmuls at
    2-3x duration.  Warmup dummies are staged with partial-semaphore
    re-syncs so they flush directly into mm1.
  - mm1/mm2 run at peak when fed (216 ns per 128x512 matmul; mm2 K=64
    window pairs run 2x concurrent in different PE row groups).  PE floor
    ~13.9 us; input supply binds mm1's 3rd k-chunk (~0.7 us stall).

Schedule: input split over both HWDGE rings exactly as v1 (A: k0, k2,
xB(k0k1), wo-A; B: k1, k3, xB(k2k3), wo-B).  mm1 half A k-outer, half B
m-outer (gelu-B tiles unlock every 0.9 us).  Output DRAM partition-major
[128, 8win, 512]; window pairs go out as full-partition 2KB-line pieces,
(w0,w1)/(w4,w5) on the sync ring, (w2,w3)/(w6,w7) on the scalar ring (a
64-partition split across rings was tried and is NOT faster: each
partition half maps to half the SDMA engines).  PSUM casts: Vector does
w0-w3, w5, w7; Scalar does w4, w6 so the last pair needs no cross-engine
wake for w6.
"""

import sys

sys.path.insert(0, "/opt/trn_rl_repo")

import numpy as np
import ml_dtypes

B, P_WIN, N_TOK, DIM = 2, 16, 256, 512
H_HEADS, DH = 8, 64
INNER = H_HEADS * DH  # 512
TOK_HALF = N_TOK // 2  # 128
N_CORES = 8

# Column layout of the packed input mega-tile (128 partitions, bf16):
#   [1024k : 1024k+512)    wv k-chunk;  [1024k+512 : 1024k+1024) xtA k-chunk
#   [4096:6144)   xtB : X^T token-half B, 4 k-chunks of (128, 512)
#   [6144:8192)   wo  : W_out natural, 4 tiles of (128, 512)
XTB_OFF, WO_OFF, IN_COLS = 4096, 6144, 8192

# PE warmup.  The PE clock climbs a stepped DVFS ladder (first full-width
# matmuls after idle run at 3x/2x duration, ~1.1 us lost); the climb only
# advances under REAL array activity, so a few full-size (N=512) dummies
# pre-climb it during the input DMA, sized to finish ~ at ka0's completion
# semaphore in both clock regimes (216 vs 259 ns steady slots run-to-run).
# A small-dummy pad keeps the clock up across the residual wait.
N_BIG_DUMMY = 6
SMALL_PAD = 3

_COMPILED = None


def _build_bass():
    import concourse.bass as bass
    import concourse.mybir as mybir

    dt = mybir.dt
    nc = bass.Bass()

    inp = nc.declare_dram_parameter("inp", [128, IN_COLS], dt.bfloat16, isOutput=False)
    # Partition-major output: [sbuf partition (token), window, dim] -- host
    # transposes back.  Keeps every output DMA at 2 KB contiguous per line.
    out = nc.declare_dram_parameter("out", [128, 8, 512], dt.bfloat16, isOutput=True)

    from contextlib import ExitStack

    stack = ExitStack()
    sem = lambda n: stack.enter_context(nc.semaphore(n))
    with (
        nc.sbuf_tensor([128, IN_COLS], dt.bfloat16) as mega,
        nc.sbuf_tensor([128, 4, 1024], dt.bfloat16) as g_t,
        nc.sbuf_tensor([128, 8, 512], dt.bfloat16) as out_t,
        nc.sbuf_tensor([128, 16], dt.float32) as scratch,
        nc.sbuf_tensor([128, 192], dt.bfloat16) as scratch_bf,
        nc.sbuf_tensor([128, 4, 512], dt.bfloat16) as wo_shift,
        nc.psum_tensor([128, 8, 512], dt.float32) as ps,
        stack,
        nc.Block() as block,
    ):
        ka_sems = [sem(f"ka{k}_sem") for k in range(4)]
        xba_sem, xbb_sem = sem("xba_sem"), sem("xbb_sem")
        woa_sem, wob_sem = sem("woa_sem"), sem("wob_sem")
        dmao_sem = sem("dmao_sem")
        pe1_sem, pe2_sem = sem("pe1_sem"), sem("pe2_sem")
        act_sem, dve_sem = sem("act_sem"), sem("dve_sem")

        def xt_sl(k, nch):
            off = (1024 * k + 512) if nch == 0 else (XTB_OFF + 512 * k)
            return mega[:, off : off + 512]

        def wv_sl(k, m):
            return mega[:, 1024 * k + 128 * m : 1024 * k + 128 * m + 128]

        def wo_sl(h, jp):
            if jp == 64 * (h % 2):  # natural position in the loaded W_out
                o = WO_OFF + 512 * (h // 2)
                return mega[jp : jp + 64, o : o + 512]
            return wo_shift[jp : jp + 64, h // 2, :]

        def in_dma(eng, lo, hi, s):
            eng.dma_start(out=mega[:, lo:hi], in_=inp[:, lo:hi]).then_inc(s, 16)

        @block.sync
        def _(sync):
            in_dma(sync, 0, 1024, ka_sems[0])
            in_dma(sync, 2048, 3072, ka_sems[2])
            in_dma(sync, XTB_OFF, XTB_OFF + 1024, xba_sem)
            in_dma(sync, WO_OFF, WO_OFF + 1024, woa_sem)
            # Output pairs (w0,w1) and (w4,w5) on this ring
            sync.wait_ge(dve_sem, 8)
            sync.dma_start(out=out[:, 0:2, :], in_=out_t[:, 0:2, :]).then_inc(
                dmao_sem, 16
            )
            sync.wait_ge(act_sem, 9)
            sync.wait_ge(dve_sem, 11)
            sync.dma_start(out=out[:, 4:6, :], in_=out_t[:, 4:6, :]).then_inc(
                dmao_sem, 16
            )
            sync.wait_ge(dmao_sem, 64)

        @block.scalar
        def _(scalar):
            in_dma(scalar, 1024, 2048, ka_sems[1])
            in_dma(scalar, 3072, 4096, ka_sems[3])
            in_dma(scalar, XTB_OFF + 1024, WO_OFF, xbb_sem)
            in_dma(scalar, WO_OFF + 1024, IN_COLS, wob_sem)
            # Pre-warm the gelu spline table during the input DMA.
            scalar.wait_ge(dve_sem, 1)
            nc.scalar.activation(
                scratch[:, 8:], scratch[:, :8], mybir.ActivationFunctionType.Gelu
            )
            # gelu tiles: half A (m0-3) then half B (m0-3)
            for i in range(8):
                nch, m = i // 4, i % 4
                scalar.wait_ge(pe1_sem, i + 1)
                nc.scalar.activation(
                    g_t[:, m, 512 * nch : 512 * nch + 512],
                    ps[:, 4 * nch + m, :],
                    mybir.ActivationFunctionType.Gelu,
                ).then_inc(act_sem, 1)
            # Output pairs (w2,w3) and (w6,w7) on this ring, interleaved with
            # the w4/w6 casts this engine owns.  w6's cast is local, so the
            # last pair's issue avoids a cross-engine semaphore wake.
            scalar.wait_ge(dve_sem, 10)
            scalar.dma_start(out=out[:, 2:4, :], in_=out_t[:, 2:4, :]).then_inc(
                dmao_sem, 16
            )
            scalar.wait_ge(pe2_sem, 5)
            nc.scalar.copy(out_t[:, 4, :], ps[:, 4, :]).then_inc(act_sem, 1)
            scalar.wait_ge(pe2_sem, 7)
            nc.scalar.copy(out_t[:, 6, :], ps[:, 6, :]).then_inc(act_sem, 1)
            scalar.wait_ge(dve_sem, 12)
            scalar.dma_start(out=out[:, 6:8, :], in_=out_t[:, 6:8, :]).then_inc(
                dmao_sem, 16
            )

        @block.vector
        def _(vector):
            nc.vector.memset(scratch[:, :8], 0.0).then_inc(dve_sem, 1)
            nc.vector.memset(scratch_bf[:], 0.0).then_inc(dve_sem, 1)
            # Build wo_shift: each W_out row-half copied to the OPPOSITE
            # partition half (DVE 33-64ch ops may write either half).
            for piece, wsem in ((0, woa_sem), (1, wob_sem)):
                vector.wait_ge(wsem, 16)
                o = WO_OFF + 1024 * piece
                src_lo = mega[0:64, o : o + 1024].rearrange("p (c t) -> p c t", c=2)
                src_hi = mega[64:128, o : o + 1024].rearrange("p (c t) -> p c t", c=2)
                d0, d1 = 2 * piece, 2 * piece + 2
                nc.vector.tensor_copy(wo_shift[64:128, d0:d1, :], src_lo).then_inc(dve_sem, 1)
                nc.vector.tensor_copy(wo_shift[0:64, d0:d1, :], src_hi).then_inc(dve_sem, 1)
            # PSUM -> SBUF bf16 casts: w0-w3, then w5 and w7 (w4/w6 on scalar)
            for pl in (0, 1, 2, 3):
                vector.wait_ge(pe2_sem, pl + 1)
                nc.vector.tensor_copy(out_t[:, pl, :], ps[:, pl, :]).then_inc(dve_sem, 1)
            vector.wait_ge(pe2_sem, 6)
            nc.vector.tensor_copy(out_t[:, 5, :], ps[:, 5, :]).then_inc(dve_sem, 1)
            vector.wait_ge(pe2_sem, 8)
            nc.vector.tensor_copy(out_t[:, 7, :], ps[:, 7, :]).then_inc(dve_sem, 1)

        @block.tensor
        def _(tensor):
            # PE clock warmup (see N_BIG_DUMMY note above).  The big dummies
            # read out_t, which is garbage until the output casts much later.
            tensor.wait_ge(dve_sem, 2)
            for _ in range(N_BIG_DUMMY):
                nc.tensor.matmul(
                    ps[:, 0, :],
                    lhsT=out_t[:, 1, 0:128],
                    rhs=out_t[:, 0, :],
                    start=True,
                    stop=True,
                    skip_group_check=True,
                )
            tensor.wait_ge(ka_sems[0], 8)
            for _ in range(SMALL_PAD):
                nc.tensor.matmul(
                    ps[0:64, 0, 0:64],
                    lhsT=scratch_bf[:, 0:64],
                    rhs=scratch_bf[:, 64:128],
                    start=True,
                    stop=True,
                    skip_group_check=True,
                )
            tensor.wait_ge(ka_sems[0], 16)
            # mm1 half A (k-outer): bank m <- sum_k wv[k,m]^T @ xA[k]
            for k in range(4):
                if k > 0:
                    tensor.wait_ge(ka_sems[k], 16)
                for m in range(4):
                    mm = nc.tensor.matmul(
                        ps[:, m, :],
                        lhsT=wv_sl(k, m),
                        rhs=xt_sl(k, 0),
                        start=(k == 0),
                        stop=(k == 3),
                        skip_group_check=True,
                    )
                    if k == 3:
                        mm.then_inc(pe1_sem, 1)
            # mm1 half B (m-outer): bank 4+m completes every ~0.9us
            tensor.wait_ge(xba_sem, 16)
            for m in range(4):
                for k in range(4):
                    if m == 0 and k == 2:
                        tensor.wait_ge(xbb_sem, 16)
                    mm = nc.tensor.matmul(
                        ps[:, 4 + m, :],
                        lhsT=wv_sl(k, m),
                        rhs=xt_sl(k, 1),
                        start=(k == 0),
                        stop=(k == 3),
                        skip_group_check=True,
                    )
                    if k == 3:
                        mm.then_inc(pe1_sem, 1)
            # mm2: window pairs (2pp, 2pp+1) interleaved across PE row groups
            for pp in range(4):
                tensor.wait_ge(act_sem, 2 * pp + 2)  # banks freed + g-A tile pp
                if pp == 0:
                    tensor.wait_ge(dve_sem, 4)  # wo_shift half A built
                for hh in range(8):
                    if hh == 4:
                        tensor.wait_ge(act_sem, 5 + pp)  # g tile pp, half B
                        if pp == 0:
                            tensor.wait_ge(dve_sem, 6)  # wo_shift half B built
                    for pl in (2 * pp, 2 * pp + 1):
                        jp = 64 * (pl % 2)
                        mm = nc.tensor.matmul(
                            ps[:, pl, :],
                            lhsT=g_t[jp : jp + 64, pp, 128 * hh : 128 * hh + 128],
                            rhs=wo_sl(hh, jp),
                            start=(hh == 0),
                            stop=(hh == 7),
                            skip_group_check=True,
                        )
                        if hh == 7:
                            mm.then_inc(pe2_sem, 1)

    return nc


def _shard_inputs(x, W_qkv, W_out):
    bf16 = ml_dtypes.bfloat16
    W_v = np.ascontiguousarray(W_qkv[:, 2 * INNER : 3 * INNER]).astype(bf16)
    wv_chunks = W_v.reshape(4, 128, 512).transpose(1, 0, 2)  # (128, 4, 512)
    wo_part = (
        W_out.astype(bf16).reshape(4, 128, 512).transpose(1, 0, 2).reshape(128, 2048)
    )
    in_maps = []
    for c in range(N_CORES):
        b, q, t = c // 4, (c // 2) % 2, c % 2
        xs = x[b, q::2, TOK_HALF * t : TOK_HALF * (t + 1), :]  # (8, 128, 512)
        xt = np.ascontiguousarray(xs.transpose(2, 0, 1).reshape(512, 1024)).astype(bf16)
        xt4 = xt.reshape(4, 128, 1024)
        xtA = xt4[:, :, :512].transpose(1, 0, 2)  # (128, 4, 512)
        xtB = xt4[:, :, 512:].transpose(1, 0, 2).reshape(128, 2048)
        front = np.concatenate([wv_chunks, xtA], axis=2).reshape(128, 4096)
        mega = np.concatenate([front, xtB, wo_part], axis=1)
        in_maps.append({"inp": np.ascontiguousarray(mega)})
    return in_maps


def _assemble(results, b_out):
    out = np.empty((B, P_WIN, N_TOK, DIM), dtype=np.float32)
    for c in range(N_CORES):
        b, q, t = c // 4, (c // 2) % 2, c % 2
        r = np.asarray(results[c]["out"]).astype(np.float32)  # (128, 8, 512)
        out[b, 8 * q : 8 * q + 8, TOK_HALF * t : TOK_HALF * (t + 1), :] = r.transpose(
            1, 0, 2
        )
    out += b_out.astype(np.float32)
    return out


def _run(inputs, trace=False, trace_cores=None):
    global _COMPILED
    from concourse.bass_utils import run_bass_kernel_spmd

    if _COMPILED is None:
        _COMPILED = _build_bass()
    nc = _COMPILED
    in_maps = _shard_inputs(
        np.asarray(inputs["x"]), np.asarray(inputs["W_qkv"]), np.asarray(inputs["W_out"])
    )
    res = run_bass_kernel_spmd(
        nc, in_maps, core_ids=list(range(N_CORES)), trace=trace, trace_cores=trace_cores
    )
    out = _assemble(res.results, np.asarray(inputs["b_out"]))
    return out, res


def kernel(x, grad, W_qkv, W_out, b_out):
    out, _ = _run(dict(x=x, grad=grad, W_qkv=W_qkv, W_out=W_out, b_out=b_out))
    return out


# revision 29
# speedup vs baseline: 1.1432x; 1.1432x over previous
"""Trainium2 Bass kernel for nn_Attention_38130719654026 (sparse_attention).

The reference collapses exactly (verified to 9e-8 rel err): the top-k gather
broadcasts kv over the topk axis, so attention logits are constant along it,
softmax is uniform, and attn @ v_sel returns v unchanged.  grad/q/k are dead.
What remains is:

    g   = gelu(x @ W_v)              with W_v = W_qkv[:, 1024:1536]
    y[b, P, n, 64H:64H+64] = g[b, 2H + P//8, n, 64*(P%8) : 64*(P%8)+64]
    out = y @ W_out + b_out

Sharding (8 cores, no collectives): core c -> (batch b = c//4,
window parity q = (c//2)%2, token half t = c%2).

Measured HW model (perfetto traces across 4 kernel revisions):
  - each HWDGE queue sustains ~160 GB/s for >=2KB lines; sub-2KB lines are
    descriptor-bound (~12.6 ns/desc).  Every full-partition piece costs
    >=128 descs.  The 16 SDMA engines round-robin between ACTIVE queues at
    packet granularity, so a third queue (gpsimd SWDGE ~46 GB/s) or even a
    1-desc warmup DMA slows the critical stream -- both tested and reverted.
  - per-piece completion semaphores straggle 0.4-1.6 us behind the bulk of
    the data (16 per-engine increments, last ones delayed by engine
    round-robin).
  - the PE re-enters clock gating after ~1 us idle, then runs ~6 matmuls at
    2-3x duration.  Warmup dummies are staged with partial-semaphore
    re-syncs so they flush directly into mm1.
  - mm1/mm2 run at peak when fed (216 ns per 128x512 matmul; mm2 K=64
    window pairs run 2x concurrent in different PE row groups).  PE floor
    ~13.9 us; input supply binds mm1's 3rd k-chunk (~0.7 us stall).

Schedule: input split over both HWDGE rings exactly as v1 (A: k0, k2,
xB(k0k1), wo-A; B: k1, k3, xB(k2k3), wo-B).  mm1 half A k-outer, half B
m-outer (gelu-B tiles unlock every 0.9 us).  Output DRAM partition-major
[128, 8win, 512]; window pairs go out as full-partition 2KB-line pieces,
(w0,w1)/(w4,w5) on the sync ring, (w2,w3)/(w6,w7) on the scalar ring (a
64-partition split across rings was tried and is NOT faster: each
partition half maps to half the SDMA engines).  PSUM casts: Vector does
w0-w3, w5, w7; Scalar does w4, w6 so the last pair needs no cross-engine
wake for w6.
"""

import sys

sys.path.insert(0, "/opt/trn_rl_repo")

import numpy as np
import ml_dtypes

B, P_WIN, N_TOK, DIM = 2, 16, 256, 512
H_HEADS, DH = 8, 64
INNER = H_HEADS * DH  # 512
TOK_HALF = N_TOK // 2  # 128
N_CORES = 8

# Column layout of the packed input mega-tile (128 partitions, bf16):
#   [1024k : 1024k+512)    wv k-chunk;  [1024k+512 : 1024k+1024) xtA k-chunk
#   [4096:6144)   xtB : X^T token-half B, 4 k-chunks of (128, 512)
#   [6144:8192)   wo  : W_out natural, 4 tiles of (128, 512)
XTB_OFF, WO_OFF, IN_COLS = 4096, 6144, 8192

# PE warmup: small matmuls until ka0's completion semaphore.  The first ~5
# real matmuls run at quantized 3x/2x duration regardless (a stepped DVFS
# ladder that only climbs under sustained full-width activity and RESETS
# across any pause -- pre-climbing it with full-size dummies was measured
# and fails: the ladder decays during the final semaphore wait and mm1
# ramps again anyway, net -1 us).  Two stages with a partial-semaphore
# re-sync keep the flush into mm1 tight in BOTH clock regimes (a fixed
# count overshoots ~1 us when the PE clock is low, since the DMA side is
# clock-independent).
DUMMY_STAGES = [(40, 1), (10, 16)]  # (n_dummies, then wait ka0 >= v)

_COMPILED = None


def _build_bass():
    import concourse.bass as bass
    import concourse.mybir as mybir

    dt = mybir.dt
    nc = bass.Bass()

    inp = nc.declare_dram_parameter("inp", [128, IN_COLS], dt.bfloat16, isOutput=False)
    # Partition-major output: [sbuf partition (token), window, dim] -- host
    # transposes back.  Keeps every output DMA at 2 KB contiguous per line.
    out = nc.declare_dram_parameter("out", [128, 8, 512], dt.bfloat16, isOutput=True)

    from contextlib import ExitStack

    stack = ExitStack()
    sem = lambda n: stack.enter_context(nc.semaphore(n))
    with (
        nc.sbuf_tensor([128, IN_COLS], dt.bfloat16) as mega,
        nc.sbuf_tensor([128, 4, 1024], dt.bfloat16) as g_t,
        nc.sbuf_tensor([128, 8, 512], dt.bfloat16) as out_t,
        nc.sbuf_tensor([128, 16], dt.float32) as scratch,
        nc.sbuf_tensor([128, 192], dt.bfloat16) as scratch_bf,
        nc.sbuf_tensor([128, 4, 512], dt.bfloat16) as wo_shift,
        nc.psum_tensor([128, 8, 512], dt.float32) as ps,
        stack,
        nc.Block() as block,
    ):
        ka_sems = [sem(f"ka{k}_sem") for k in range(4)]
        xba_sem, xbb_sem = sem("xba_sem"), sem("xbb_sem")
        woa_sem, wob_sem = sem("woa_sem"), sem("wob_sem")
        dmao_sem = sem("dmao_sem")
        pe1_sem, pe2_sem = sem("pe1_sem"), sem("pe2_sem")
        act_sem, dve_sem = sem("act_sem"), sem("dve_sem")

        def xt_sl(k, nch):
            off = (1024 * k + 512) if nch == 0 else (XTB_OFF + 512 * k)
            return mega[:, off : off + 512]

        def wv_sl(k, m):
            return mega[:, 1024 * k + 128 * m : 1024 * k + 128 * m + 128]

        def wo_sl(h, jp):
            if jp == 64 * (h % 2):  # natural position in the loaded W_out
                o = WO_OFF + 512 * (h // 2)
                return mega[jp : jp + 64, o : o + 512]
            return wo_shift[jp : jp + 64, h // 2, :]

        def in_dma(eng, lo, hi, s):
            eng.dma_start(out=mega[:, lo:hi], in_=inp[:, lo:hi]).then_inc(s, 16)

        @block.sync
        def _(sync):
            in_dma(sync, 0, 1024, ka_sems[0])
            in_dma(sync, 2048, 3072, ka_sems[2])
            in_dma(sync, XTB_OFF, XTB_OFF + 1024, xba_sem)
            in_dma(sync, WO_OFF, WO_OFF + 1024, woa_sem)
            # Output pairs (w0,w1) and (w4,w5) on this ring
            sync.wait_ge(dve_sem, 8)
            sync.dma_start(out=out[:, 0:2, :], in_=out_t[:, 0:2, :]).then_inc(
                dmao_sem, 16
            )
            sync.wait_ge(act_sem, 9)
            sync.wait_ge(dve_sem, 11)
            sync.dma_start(out=out[:, 4:6, :], in_=out_t[:, 4:6, :]).then_inc(
                dmao_sem, 16
            )
            sync.wait_ge(dmao_sem, 64)

        @block.scalar
        def _(scalar):
            in_dma(scalar, 1024, 2048, ka_sems[1])
            in_dma(scalar, 3072, 4096, ka_sems[3])
            in_dma(scalar, XTB_OFF + 1024, WO_OFF, xbb_sem)
            in_dma(scalar, WO_OFF + 1024, IN_COLS, wob_sem)
            # Pre-warm the gelu spline table during the input DMA.
            scalar.wait_ge(dve_sem, 1)
            nc.scalar.activation(
                scratch[:, 8:], scratch[:, :8], mybir.ActivationFunctionType.Gelu
            )
            # gelu tiles: half A (m0-3) then half B (m0-3)
            for i in range(8):
                nch, m = i // 4, i % 4
                scalar.wait_ge(pe1_sem, i + 1)
                nc.scalar.activation(
                    g_t[:, m, 512 * nch : 512 * nch + 512],
                    ps[:, 4 * nch + m, :],
                    mybir.ActivationFunctionType.Gelu,
                ).then_inc(act_sem, 1)
            # Output pairs (w2,w3) and (w6,w7) on this ring, interleaved with
            # the w4/w6 casts this engine owns.  w6's cast is local, so the
            # last pair's issue avoids a cross-engine semaphore wake.
            scalar.wait_ge(dve_sem, 10)
            scalar.dma_start(out=out[:, 2:4, :], in_=out_t[:, 2:4, :]).then_inc(
                dmao_sem, 16
            )
            scalar.wait_ge(pe2_sem, 5)
            nc.scalar.copy(out_t[:, 4, :], ps[:, 4, :]).then_inc(act_sem, 1)
            scalar.wait_ge(pe2_sem, 7)
            nc.scalar.copy(out_t[:, 6, :], ps[:, 6, :]).then_inc(act_sem, 1)
            scalar.wait_ge(dve_sem, 12)
            scalar.dma_start(out=out[:, 6:8, :], in_=out_t[:, 6:8, :]).then_inc(
                dmao_sem, 16
            )

        @block.vector
        def _(vector):
            nc.vector.memset(scratch[:, :8], 0.0).then_inc(dve_sem, 1)
            nc.vector.memset(scratch_bf[:], 0.0).then_inc(dve_sem, 1)
            # Build wo_shift: each W_out row-half copied to the OPPOSITE
            # partition half (DVE 33-64ch ops may write either half).
            for piece, wsem in ((0, woa_sem), (1, wob_sem)):
                vector.wait_ge(wsem, 16)
                o = WO_OFF + 1024 * piece
                src_lo = mega[0:64, o : o + 1024].rearrange("p (c t) -> p c t", c=2)
                src_hi = mega[64:128, o : o + 1024].rearrange("p (c t) -> p c t", c=2)
                d0, d1 = 2 * piece, 2 * piece + 2
                nc.vector.tensor_copy(wo_shift[64:128, d0:d1, :], src_lo).then_inc(dve_sem, 1)
                nc.vector.tensor_copy(wo_shift[0:64, d0:d1, :], src_hi).then_inc(dve_sem, 1)
            # PSUM -> SBUF bf16 casts: w0-w3, then w5 and w7 (w4/w6 on scalar)
            for pl in (0, 1, 2, 3):
                vector.wait_ge(pe2_sem, pl + 1)
                nc.vector.tensor_copy(out_t[:, pl, :], ps[:, pl, :]).then_inc(dve_sem, 1)
            vector.wait_ge(pe2_sem, 6)
            nc.vector.tensor_copy(out_t[:, 5, :], ps[:, 5, :]).then_inc(dve_sem, 1)
            vector.wait_ge(pe2_sem, 8)
            nc.vector.tensor_copy(out_t[:, 7, :], ps[:, 7, :]).then_inc(dve_sem, 1)

        @block.tensor
        def _(tensor):
            # PE clock warmup (small, low-power; see DUMMY_STAGES note above).
            tensor.wait_ge(dve_sem, 2)
            for n_dum, ka_target in DUMMY_STAGES:
                for _ in range(n_dum):
                    nc.tensor.matmul(
                        ps[0:64, 0, 0:64],
                        lhsT=scratch_bf[:, 0:64],
                        rhs=scratch_bf[:, 64:128],
                        start=True,
                        stop=True,
                        skip_group_check=True,
                    )
                tensor.wait_ge(ka_sems[0], ka_target)
            # mm1 half A (k-outer): bank m <- sum_k wv[k,m]^T @ xA[k]
            for k in range(4):
                if k > 0:
                    tensor.wait_ge(ka_sems[k], 16)
                for m in range(4):
                    mm = nc.tensor.matmul(
                        ps[:, m, :],
                        lhsT=wv_sl(k, m),
                        rhs=xt_sl(k, 0),
                        start=(k == 0),
                        stop=(k == 3),
                        skip_group_check=True,
                    )
                    if k == 3:
                        mm.then_inc(pe1_sem, 1)
            # mm1 half B (m-outer): bank 4+m completes every ~0.9us
            tensor.wait_ge(xba_sem, 16)
            for m in range(4):
                for k in range(4):
                    if m == 0 and k == 2:
                        tensor.wait_ge(xbb_sem, 16)
                    mm = nc.tensor.matmul(
                        ps[:, 4 + m, :],
                        lhsT=wv_sl(k, m),
                        rhs=xt_sl(k, 1),
                        start=(k == 0),
                        stop=(k == 3),
                        skip_group_check=True,
                    )
                    if k == 3:
                        mm.then_inc(pe1_sem, 1)
            # mm2: window pairs (2pp, 2pp+1) interleaved across PE row groups
            for pp in range(4):
                tensor.wait_ge(act_sem, 2 * pp + 2)  # banks freed + g-A tile pp
                if pp == 0:
                    tensor.wait_ge(dve_sem, 4)  # wo_shift half A built
                for hh in range(8):
                    if hh == 4:
                        tensor.wait_ge(act_sem, 5 + pp)  # g tile pp, half B
                        if pp == 0:
                            tensor.wait_ge(dve_sem, 6)  # wo_shift half B built
                    for pl in (2 * pp, 2 * pp + 1):
                        jp = 64 * (pl % 2)
                        mm = nc.tensor.matmul(
                            ps[:, pl, :],
                            lhsT=g_t[jp : jp + 64, pp, 128 * hh : 128 * hh + 128],
                            rhs=wo_sl(hh, jp),
                            start=(hh == 0),
                            stop=(hh == 7),
                            skip_group_check=True,
                        )
                        if hh == 7:
                            mm.then_inc(pe2_sem, 1)

    return nc


def _shard_inputs(x, W_qkv, W_out):
    bf16 = ml_dtypes.bfloat16
    W_v = np.ascontiguousarray(W_qkv[:, 2 * INNER : 3 * INNER]).astype(bf16)
    wv_chunks = W_v.reshape(4, 128, 512).transpose(1, 0, 2)  # (128, 4, 512)
    wo_part = (
        W_out.astype(bf16).reshape(4, 128, 512).transpose(1, 0, 2).reshape(128, 2048)
    )
    in_maps = []
    for c in range(N_CORES):
        b, q, t = c // 4, (c // 2) % 2, c % 2
        xs = x[b, q::2, TOK_HALF * t : TOK_HALF * (t + 1), :]  # (8, 128, 512)
        xt = np.ascontiguousarray(xs.transpose(2, 0, 1).reshape(512, 1024)).astype(bf16)
        xt4 = xt.reshape(4, 128, 1024)
        xtA = xt4[:, :, :512].transpose(1, 0, 2)  # (128, 4, 512)
        xtB = xt4[:, :, 512:].transpose(1, 0, 2).reshape(128, 2048)
        front = np.concatenate([wv_chunks, xtA], axis=2).reshape(128, 4096)
        mega = np.concatenate([front, xtB, wo_part], axis=1)
        in_maps.append({"inp": np.ascontiguousarray(mega)})
    return in_maps


def _assemble(results, b_out):
    out = np.empty((B, P_WIN, N_TOK, DIM), dtype=np.float32)
    for c in range(N_CORES):
        b, q, t = c // 4, (c // 2) % 2, c % 2
        r = np.asarray(results[c]["out"]).astype(np.float32)  # (128, 8, 512)
        out[b, 8 * q : 8 * q + 8, TOK_HALF * t : TOK_HALF * (t + 1), :] = r.transpose(
            1, 0, 2
        )
    out += b_out.astype(np.float32)
    return out


def _run(inputs, trace=False, trace_cores=None):
    global _COMPILED
    from concourse.bass_utils import run_bass_kernel_spmd

    if _COMPILED is None:
        _COMPILED = _build_bass()
    nc = _COMPILED
    in_maps = _shard_inputs(
        np.asarray(inputs["x"]), np.asarray(inputs["W_qkv"]), np.asarray(inputs["W_out"])
    )
    res = run_bass_kernel_spmd(
        nc, in_maps, core_ids=list(range(N_CORES)), trace=trace, trace_cores=trace_cores
    )
    out = _assemble(res.results, np.asarray(inputs["b_out"]))
    return out, res


def kernel(x, grad, W_qkv, W_out, b_out):
    out, _ = _run(dict(x=x, grad=grad, W_qkv=W_qkv, W_out=W_out, b_out=b_out))
    return out
